# revision 30
# baseline (speedup 1.0000x reference)
"""Trainium2 Bass kernel for a 3-layer TransformerConv GNN (nn_EncoderTransformerConv).

v2 — flat cross-layer pipeline keyed on keeping the GpSimd gather-descriptor
engine (the serial bottleneck, ~4.8ns per scanned edge slot) busy:

  - Host: nodes are re-assigned to (core, block) bins so per-(core, half,
    block) edge counts are flattened to <=1024 with overflow concentrated in
    the top block index -> tile count drops ~11% (and with it gather-ucode
    time, S/ST bytes, and DVE work).
  - Gathers are issued as SWDGE prepare_only descriptors with a PREP_D-deep
    trigger pipeline: descriptor generation (the Q7 cost) runs ahead while
    triggers carry the kv-table data dependency.
  - Next-layer phase A (kv table recompute) is interleaved into this layer's
    phase B instruction stream: h^T is all-gathered in 7 column chunks as
    dst-block epilogues complete, and kv chunks for the low half are computed
    during the tail of phase B so the next layer's first gather trigger fires
    within ~50us of the last epilogue. kv tables are double-buffered by layer
    parity so writes never WAR against in-flight gathers.
  - Everything else (one-hot fp8 q-broadcast / segmented-reduce matmuls,
    folded biases, bf16 kv rows) follows the v1 design.
"""
import os
import sys
from collections import defaultdict

sys.path.insert(0, "/opt/trn_rl_repo")

import ml_dtypes
import numpy as np

import concourse.bass as bass
import concourse.bacc as bacc
import concourse.mybir as mybir
import concourse.tile as tile
from concourse import bass_utils, library_config
from concourse.masks import make_identity

F32 = mybir.dt.float32
BF16 = mybir.dt.bfloat16
FP8 = mybir.dt.float8e4
I16 = mybir.dt.int16
AF = mybir.ActivationFunctionType
OP = mybir.AluOpType
BNP = ml_dtypes.bfloat16
F8NP = ml_dtypes.float8_e4m3

N_QUEUES = 4          # SWDGE queues; gathers round-robin across them
SPEC = dict(N=50000, E=800000, D_IN=128, HID=64, H=2, M=8)
TILE_E = 128          # edges per tile
CHUNK_T = 20          # tiles per edge-phase chunk
BLK = 128             # dst nodes per block
KVG_BUFS = 4
BAL_LIMIT = 8 * TILE_E   # per-(core,half,block) edge-count target (8 tiles)
COLL_MARGIN = 3       # B-chunks between a block's data chunk and its coll issue
KVHI_START, KVHI_PER = 1, 3  # hi-half kv chunks: 3 per B-chunk from chunk 1


def _derive(cfg):
    d = dict(cfg)
    d["C"] = d["HID"]
    d["F"] = d["H"] * d["C"]            # 128 = q/k/v width
    d["WC"] = 3 * d["F"] + d["HID"]     # 448 packed k|v|q|s
    d["KV"] = 2 * d["F"]                # 256
    d["NPC_REAL"] = d["N"] // d["M"]
    d["NBLK"] = -(-d["NPC_REAL"] // BLK)
    d["NPC"] = d["NBLK"] * BLK
    d["NPAD"] = d["M"] * d["NPC"]
    d["HALF"] = d["NPAD"] // 2
    pa = 1
    for c in range(1, 9):
        if d["NBLK"] % c == 0:
            pa = c
    d["PA_CHUNK"] = pa                  # 7
    d["RANK_CH"] = d["NBLK"] // pa      # 7 phase-A columns per rank
    d["RHSW"] = d["H"] + d["F"]         # 130 = denom cols + exv cols
    return d


def _wrap_idx(a):
    """[M, n] int -> wrapped idx layout [M, 128, n//16] (16-partition wrap,
    replicated to 8 GPSIMD core groups)."""
    Mn, n = a.shape
    w = a.reshape(Mn, n // 16, 16).transpose(0, 2, 1)
    return np.ascontiguousarray(np.tile(w, (1, 8, 1))).astype(np.int16)


def _balance_core(dlo, dhi, nblk, cap, limit):
    """Assign nodes (with per-half in-degrees dlo/dhi) to nblk bins of <=cap
    nodes, flattening both half-sums to <=limit; overflow concentrates in the
    highest bin labels (so tile overflow aligns across cores)."""
    n = len(dlo)
    order = np.argsort(-(dlo + dhi), kind="stable")
    slo = np.zeros(nblk, np.int64)
    shi = np.zeros(nblk, np.int64)
    cnt = np.zeros(nblk, np.int64)
    assign = np.empty(n, np.int64)
    cnt_soft = n / nblk
    for v in order:
        a, b = int(dlo[v]), int(dhi[v])
        ok = (cnt < cap) & (slo + a <= limit) & (shi + b <= limit)
        if ok.any():
            # keep degree AND count pressure flat so the feasible set stays
            # open; the max-term steers away from whichever binds first
            score = np.maximum(
                np.maximum(slo + a, shi + b) / limit,
                (cnt + 1) / cnt_soft)
            score[~ok] = 1e18
            bin_ = int(np.argmin(score))
        else:
            load = (slo + shi).astype(np.float64)
            load[cnt >= cap] = -1e18
            bin_ = int(np.argmax(load))
        assign[v] = bin_
        slo[bin_] += a
        shi[bin_] += b
        cnt[bin_] += 1
    # repair: concentrate residual overshoot into one overflow bin via
    # count-preserving swaps (move a heavy node out of an over-limit bin,
    # a light node in from the overflow bin)
    T = int(np.argmax(slo + shi))
    nodes_by_bin = [list(np.nonzero(assign == b)[0]) for b in range(nblk)]
    for B in range(nblk):
        if B == T:
            continue
        guard = 0
        while (slo[B] > limit or shi[B] > limit) and guard < 200:
            guard += 1
            key = dlo if (slo[B] - limit) >= (shi[B] - limit) else dhi
            vb = max(nodes_by_bin[B], key=lambda v: (key[v], dlo[v] + dhi[v]))
            wt = min(nodes_by_bin[T], key=lambda v: dlo[v] + dhi[v])
            for v, frm, to in ((vb, B, T), (wt, T, B)):
                nodes_by_bin[frm].remove(v)
                nodes_by_bin[to].append(v)
                assign[v] = to
                slo[frm] -= dlo[v]; shi[frm] -= dhi[v]; cnt[frm] -= 1
                slo[to] += dlo[v]; shi[to] += dhi[v]; cnt[to] += 1
    # overflow (most-loaded) bins land at the LOWEST labels so the last
    # blocks processed — which gate the layer-boundary collective — are light
    key = np.maximum(slo, shi)
    relab = np.empty(nblk, np.int64)
    relab[np.argsort(-key, kind="stable")] = np.arange(nblk)
    return relab[assign]


def _prep(x, edge_index, weights, d):
    """Host-side preprocessing -> (in_maps, meta)."""
    M, NPC_REAL, NPC, NPAD, HALF, NBLK = (
        d["M"], d["NPC_REAL"], d["NPC"], d["NPAD"], d["HALF"], d["NBLK"])
    N, D_IN = d["N"], d["D_IN"]

    src = np.asarray(edge_index[0]).astype(np.int64)
    dst = np.asarray(edge_index[1]).astype(np.int64)

    # ---- balanced node -> (core, block) assignment ----
    core_of_node = np.arange(N) // NPC_REAL     # core split kept as-is
    lo_edge = core_of_node[src] < M // 2
    dlo_n = np.bincount(dst[lo_edge], minlength=N)
    dhi_n = np.bincount(dst[~lo_edge], minlength=N)
    pos_of_node = np.empty(N, np.int64)
    node_of_pos = np.full((M, NPC), -1, np.int64)
    for m in range(M):
        ids = np.arange(m * NPC_REAL, (m + 1) * NPC_REAL)
        blk_a = _balance_core(dlo_n[ids], dhi_n[ids], NBLK, BLK, BAL_LIMIT)
        order = np.argsort(blk_a, kind="stable")
        cnts = np.bincount(blk_a, minlength=NBLK)
        offs = np.concatenate([[0], np.cumsum(cnts)])
        ranks = np.arange(NPC_REAL) - offs[blk_a[order]]
        pos_local = blk_a[order] * BLK + ranks
        pos_of_node[ids[order]] = m * NPC + pos_local
        node_of_pos[m, pos_local] = ids[order]

    p_src = pos_of_node[src]
    p_dst = pos_of_node[dst]
    core = p_dst // NPC
    pl = p_dst - core * NPC
    blk = pl // BLK
    dloc_e = pl - blk * BLK
    half = (p_src >= HALF).astype(np.int64)

    counts = np.zeros((M, 2, NBLK), np.int64)
    np.add.at(counts, (core, half, blk), 1)
    tiles = np.maximum(1, -(-counts.max(axis=0) // TILE_E))  # [2, NBLK]
    flat_tiles = tiles.reshape(-1)
    tile_off = np.concatenate([[0], np.cumsum(flat_tiles)])
    TT = int(tile_off[-1])

    # stable-sort edges by (core, half, blk); rank within group
    key = (core * 2 + half) * NBLK + blk
    order = np.argsort(key, kind="stable")
    sk = key[order]
    new_run = np.ones(len(sk), bool)
    new_run[1:] = sk[1:] != sk[:-1]
    run_idx = np.cumsum(new_run) - 1
    starts = np.nonzero(new_run)[0]
    rank = np.arange(len(sk)) - starts[run_idx]
    grp = (half * NBLK + blk)[order]
    pos = tile_off[grp] * TILE_E + rank
    corev = core[order]

    kv_idx = np.zeros((M, TT * TILE_E), np.int64)
    dloc = np.full((M, TT * TILE_E), -1, np.int64)
    kv_idx[corev, pos] = (p_src - half * HALF)[order]
    dloc[corev, pos] = dloc_e[order]
    assert kv_idx.max() < 2 ** 15

    S = np.zeros((M, 128, TT * BLK), F8NP)
    ST = np.zeros((M, 128, TT * BLK), F8NP)
    dd = dloc.reshape(M, TT, TILE_E)
    mm, tt, pp = np.nonzero(dd >= 0)
    dv = dd[mm, tt, pp]
    S[mm, pp, tt * BLK + dv] = 1.0
    ST[mm, dv, tt * BLK + pp] = 1.0

    kv_w = _wrap_idx(kv_idx)

    # node features, transposed + padded (bf16)
    xT_pad = np.zeros((D_IN, NPAD), np.float32)
    xT_pad[:, pos_of_node] = np.asarray(x).T
    xT = xT_pad.astype(BNP)

    wt = {}
    for L in (1, 2, 3):
        W_all, b_all = weights[L]   # packed k|v|q|s by _weights_from_inputs
        if L == 1:
            wt["W1"] = W_all.astype(BNP)
            wt["b1row"] = np.ascontiguousarray(b_all[None, :].astype(BNP))
        else:
            wt[f"W{L}"] = np.concatenate(
                [W_all, b_all[None, :]], 0).astype(BNP)

    in_maps = []
    for m in range(M):
        im = dict(
            xT=np.ascontiguousarray(xT),
            xoT=np.ascontiguousarray(xT[:, m * NPC:(m + 1) * NPC]),
            kvidx=np.ascontiguousarray(kv_w[m]),
            S_in=np.ascontiguousarray(S[m]),
            ST_in=np.ascontiguousarray(ST[m]),
            **wt,
        )
        in_maps.append(im)

    # tile metadata: (half, blk, start, stop)
    meta_t = []
    for f in (0, 1):
        for b in range(NBLK):
            T = int(tiles[f, b])
            for i in range(T):
                meta_t.append((f, b, i == 0, i == T - 1))
    # chunks: runs of <= CHUNK_T tiles within each half; the tail of half 1
    # uses small chunks so the end-of-layer pipeline drains fast
    T0 = int(tiles[0].sum())
    chunks = []
    for f, lo, hi in ((0, 0, T0), (1, T0, TT)):
        t = lo
        while t < hi:
            if f == 1 and hi - t <= 30:
                nt = min(10, hi - t)
            else:
                nt = min(CHUNK_T, hi - t)
            chunks.append((t, nt, f))
            t += nt
    chunk_of_tile = np.empty(TT, np.int64)
    for j, (t0, nt, f) in enumerate(chunks):
        chunk_of_tile[t0:t0 + nt] = j
    # last tile (global) of each half-1 block, for the coll schedule
    blk_end1 = {}
    for b in range(NBLK):
        blk_end1[b] = int(tile_off[NBLK + b] + tiles[1, b] - 1)
    return in_maps, dict(TT=TT, meta=meta_t, chunks=chunks, tiles=tiles,
                         chunk_of_tile=chunk_of_tile, blk_end1=blk_end1,
                         node_of_pos=node_of_pos)


def build_module(d, meta):
    TT, chunks, tmeta = meta["TT"], meta["chunks"], meta["meta"]
    chunk_of_tile, blk_end1 = meta["chunk_of_tile"], meta["blk_end1"]
    M, NPC, NPAD, HALF, NBLK, PA_CHUNK, RANK_CH = (
        d["M"], d["NPC"], d["NPAD"], d["HALF"], d["NBLK"], d["PA_CHUNK"],
        d["RANK_CH"])
    D_IN, F, KV, WC, HID, H, C, RHSW = (
        d["D_IN"], d["F"], d["KV"], d["WC"], d["HID"], d["H"], d["C"],
        d["RHSW"])
    # phase-A columns: 6 of 8 blocks + a final 1-block column, so the
    # layer-boundary collective + kv work (which gate the next layer's first
    # gather) are tiny
    CW = []
    b = NBLK
    while b > 0:
        w = min(8, b) if b != 9 else 8
        CW.append(w)
        b -= w
    COFF = [sum(CW[:i]) for i in range(len(CW))]
    NCOL = len(CW)
    n_ch = len(chunks)

    nc = bacc.Bacc("TRN2", target_bir_lowering=False, debug=False,
                   num_devices=M, num_swdge_queues=N_QUEUES)
    inp = {}
    for name, shape, dt in [
        ("xT", [D_IN, NPAD], BF16), ("xoT", [D_IN, NPC], BF16),
        ("W1", [D_IN, WC], BF16), ("b1row", [1, WC], BF16),
        ("W2", [HID + 1, WC], BF16), ("W3", [HID + 1, WC], BF16),
        ("kvidx", [128, TT * 8], I16),
        ("S_in", [128, TT * BLK], FP8), ("ST_in", [128, TT * BLK], FP8),
    ]:
        inp[name] = nc.dram_tensor(name, shape, dt, kind="ExternalInput")
    h_out = nc.dram_tensor("h_out", [NPC, HID], F32, kind="ExternalOutput")

    with tile.TileContext(nc) as tc:
        with tc.tile_pool(name="dram", bufs=1, space="DRAM") as dram, \
             tc.tile_pool(name="res", bufs=1) as res, \
             tc.tile_pool(name="pa", bufs=4) as pa, \
             tc.tile_pool(name="pap", bufs=2, space="PSUM") as pap, \
             tc.tile_pool(name="pb", bufs=3) as pb, \
             tc.tile_pool(name="pb1", bufs=3) as pb1, \
             tc.tile_pool(name="ep", bufs=2) as ep:

            # kv tables double-buffered by layer parity
            kvt = {par: (dram.tile([HALF, KV], BF16, name=f"kv{par}lo"),
                         dram.tile([NPAD - HALF, KV], BF16, name=f"kv{par}hi"))
                   for par in (0, 1)}
            coll_in = {ly: [dram.tile([HID + 1, CW[c] * 128], BF16,
                                      name=f"ci{ly}_{c}")
                            for c in range(NCOL)] for ly in (1, 2)}
            coll_out = {ly: [dram.tile([M * (HID + 1), CW[c] * 128], BF16,
                                       name=f"co{ly}_{c}",
                                       addr_space="Shared")
                             for c in range(NCOL)] for ly in (1, 2)}

            nc.gpsimd.load_library(library_config.mlp)

            # resident SBUF
            W1_sb = res.tile([D_IN, WC], BF16)
            b1row_sb = res.tile([1, WC], BF16)
            ones1_sb = res.tile([1, 128], BF16)
            W2_sb = res.tile([HID + 1, WC], BF16)
            W3_sb = res.tile([HID + 1, WC], BF16)
            kvidx_sb = res.tile([128, TT * 8], I16)
            q_sb = res.tile([128, NBLK * F], BF16)
            s_sb = res.tile([128, NBLK * HID], F32)
            hTown = res.tile([HID + 1, NPC], BF16)
            partA = res.tile([128, NBLK * RHSW], BF16)
            ident = res.tile([128, 128], F32)
            eps2 = res.tile([128, H], F32)

            for sb, t in ((kvidx_sb, "kvidx"), (W1_sb, "W1"),
                          (b1row_sb, "b1row"), (W2_sb, "W2"), (W3_sb, "W3")):
                nc.sync.dma_start(sb[:], inp[t].ap())
            make_identity(nc, ident[:])
            nc.vector.memset(hTown[HID:HID + 1, :], 1.0)
            nc.vector.memset(ones1_sb[:], 1.0)
            nc.vector.memset(eps2[:], H * 1e-16)

            nregs = {}
            for (t0, nt, f) in chunks:
                nn_ = nt * TILE_E
                if nn_ not in nregs:
                    nregs[nn_] = nc.gpsimd.to_reg(nn_)

            # ---------- helpers ----------
            drain_flip = [0]

            def drain(dst_ap, src_ap):
                # 2:1 scalar:vector — DVE is the hotter engine
                if drain_flip[0] % 3 != 2:
                    nc.scalar.copy(dst_ap, src_ap)
                else:
                    nc.vector.tensor_copy(dst_ap, src_ap)
                drain_flip[0] += 1

            pap_cur = [None]  # prologue uses a wider scratch PSUM pool

            def issue_qs_chunk(layer, c):
                W_sb = {2: W2_sb, 3: W3_sb}.get(layer)
                w = CW[c]
                if layer == 1:
                    la = pa.tile([D_IN, w * 128], BF16, tag="la")
                    nc.scalar.dma_start(
                        la[:], inp["xoT"].ap()[
                            :, COFF[c] * 128:(COFF[c] + w) * 128])
                for t in range(w):
                    gt = COFF[c] + t
                    ps = pap_cur[0].tile([128, KV], F32, tag="pakv",
                                         name="paqs")
                    psv = ps[:, 0:WC - KV]
                    if layer == 1:
                        nc.tensor.matmul(psv, la[:, t * 128:(t + 1) * 128],
                                         W1_sb[:, KV:WC],
                                         start=True, stop=False)
                        nc.tensor.matmul(psv, ones1_sb[:],
                                         b1row_sb[:, KV:WC],
                                         start=False, stop=True)
                    else:
                        nc.tensor.matmul(
                            psv, hTown[:, gt * 128:(gt + 1) * 128],
                            W_sb[:, KV:WC], start=True, stop=True)
                    nc.scalar.copy(q_sb[:, gt * F:(gt + 1) * F],
                                   ps[:, 0:F])
                    nc.vector.tensor_copy(
                        s_sb[:, gt * HID:(gt + 1) * HID],
                        ps[:, F:F + HID])

            def issue_qs(layer):
                for c in range(NCOL):
                    issue_qs_chunk(layer, c)

            def issue_kv_chunk(layer, r, c):
                par = layer % 2
                w = CW[c]
                if layer == 1:
                    la = pa.tile([D_IN, w * 128], BF16, tag="la")
                    col0 = r * NPC + COFF[c] * 128
                    nc.scalar.dma_start(
                        la[:], inp["xT"].ap()[:, col0:col0 + w * 128])
                    Wkv = W1_sb
                else:
                    la = pa.tile([HID + 1, w * 128], BF16, tag="la")
                    nc.scalar.dma_start(
                        la[:], coll_out[layer - 1][c][
                            r * (HID + 1):(r + 1) * (HID + 1), :])
                    Wkv = W2_sb if layer == 2 else W3_sb
                kvst = pa.tile([128, w * KV], BF16, tag="kvst")
                for t in range(w):
                    ps = pap_cur[0].tile([128, KV], F32, tag="pakv",
                                         name="pakv")
                    nc.tensor.matmul(ps[:], la[:, t * 128:(t + 1) * 128],
                                     Wkv[:, 0:KV], start=True, stop=True)
                    drain(kvst[:, t * KV:(t + 1) * KV], ps[:])
                row0 = r * NPC + COFF[c] * 128
                kv_lo, kv_hi = kvt[par]
                tgt = (kv_lo[row0:row0 + w * 128, :] if row0 < HALF else
                       kv_hi[row0 - HALF:row0 - HALF + w * 128, :])
                nc.sync.dma_start(
                    tgt.rearrange("(t p) e -> p t e", p=128),
                    kvst[:].rearrange("p (t e) -> p t e", e=KV))

            def issue_coll(layer, c):
                # input dma was issued inline at the source block's epilogue
                nc.gpsimd.collective_compute(
                    "AllGather", OP.bypass,
                    ins=[coll_in[layer][c].opt()],
                    outs=[coll_out[layer][c].opt()],
                    replica_groups=[list(range(M))])

            col_of_end_blk = {COFF[c] + CW[c] - 1: c for c in range(NCOL)}

            def do_act(act):
                if act[0] == "kv":
                    issue_kv_chunk(act[1], act[2], act[3])
                elif act[0] == "coll":
                    issue_coll(act[1], act[2])

            # ---------- layer 1 prologue (qs chunk 0 first; rest follows) ----
            # wide scratch PSUM pool: phase-B PSUM pools are not open yet, so
            # the prologue matmul->drain pipeline gets 6 banks
            with tc.tile_pool(name="pap0", bufs=6, space="PSUM") as pap0:
                pap_cur[0] = pap0
                issue_qs_chunk(1, 0)
                for c in range(NCOL):
                    if c > 0:
                        issue_qs_chunk(1, c)
                    for r in range(M // 2):
                        issue_kv_chunk(1, r, c)
            pap_cur[0] = pap

            # ---------- layer loop: phase B with interleaved next phase A ----
            with tc.tile_pool(name="pbp", bufs=2, space="PSUM") as pbp, \
                 tc.tile_pool(name="qep", bufs=2, space="PSUM") as qep, \
                 tc.tile_pool(name="epp", bufs=2, space="PSUM") as epp:
              for layer in (1, 2, 3):
                par = layer % 2
                kv_lo, kv_hi = kvt[par]

                sched = defaultdict(list)
                # hi-half kv chunks of THIS layer into early B chunks
                hi_jobs = [("kv", layer, r, c)
                           for c in range(NCOL)
                           for r in range(M // 2, M)]
                jj = KVHI_START
                while hi_jobs:
                    for _ in range(KVHI_PER):
                        if hi_jobs:
                            sched[jj].append(hi_jobs.pop(0))
                    jj += 1
                boundary = []
                if layer < 3:
                    for c in range(NCOL):
                        jc = int(chunk_of_tile[
                            blk_end1[COFF[c] + CW[c] - 1]]) + COLL_MARGIN
                        acts = [("coll", layer, c)] + [
                            ("kv", layer + 1, r, c) for r in range(M // 2)]
                        if jc < n_ch:
                            sched[jc] += acts
                        else:
                            boundary += acts

                psum_blk = {}
                for jl, (t0, nt, fhalf) in enumerate(chunks):
                    n = nt * TILE_E
                    kvg = pb.tile([128, CHUNK_T, KV], BF16, tag="kvg",
                                  bufs=5)
                    qg = pb.tile([128, CHUNK_T, F], BF16, tag="qg")
                    Sg = pb1.tile([128, CHUNK_T * BLK], FP8, tag="Sg")
                    STg = pb1.tile([128, CHUNK_T * BLK], FP8, tag="STg")
                    prod = pb1.tile([128, CHUNK_T * F], BF16, tag="prod",
                                    bufs=2)
                    alph = pb1.tile([128, CHUNK_T * H], F32, tag="alph")
                    rhs = pb.tile([128, CHUNK_T, RHSW], BF16, tag="rhs",
                                  bufs=2)

                    in_ap = kv_lo[:] if fhalf == 0 else kv_hi[:]
                    nc.gpsimd.dma_gather(
                        out_ap=kvg[:, 0:nt, :], in_ap=in_ap,
                        idxs_ap=kvidx_sb[:, t0 * 8:t0 * 8 + nt * 8],
                        num_idxs=n, num_idxs_reg=nregs[n], elem_size=KV,
                        single_packet=False, queue_num=jl % N_QUEUES)
                    nc.sync.dma_start(
                        STg[:, 0:n],
                        inp["ST_in"].ap()[:, t0 * BLK:t0 * BLK + n])
                    nc.sync.dma_start(
                        Sg[:, 0:n],
                        inp["S_in"].ap()[:, t0 * BLK:t0 * BLK + n])
                    # q at edges via one-hot matmul; PSUM drained to bf16
                    for i in range(nt):
                        b = tmeta[t0 + i][1]
                        qe = qep.tile([128, F], F32, name="qe", tag="qe")
                        nc.tensor.matmul(
                            qe[:], STg[:, i * BLK:(i + 1) * BLK],
                            q_sb[:, b * F:(b + 1) * F],
                            start=True, stop=True)
                        if i % 3 != 2:
                            nc.scalar.copy(qg[:, i, :], qe[:])
                        else:
                            nc.vector.tensor_copy(qg[:, i, :], qe[:])

                    # chunk-wide: prod = q'_e * k_e (bf16, 2x DVE mode)
                    nc.vector.tensor_tensor(
                        out=prod[:, 0:nt * F].rearrange(
                            "p (t f) -> p t f", f=F),
                        in0=qg[:, 0:nt, :], in1=kvg[:, 0:nt, 0:F], op=OP.mult)
                    nc.vector.reduce_sum(
                        alph[:, 0:nt * H],
                        prod[:, 0:nt * F].rearrange(
                            "p (th c) -> p th c", c=C),
                        axis=mybir.AxisListType.X)
                    nc.scalar.activation(
                        rhs[:, 0:nt, 0:H],
                        alph[:, 0:nt * H].rearrange("p (t h) -> p t h", h=H),
                        AF.Exp)
                    nc.vector.tensor_tensor(
                        out=rhs[:, 0:nt, H:RHSW].rearrange(
                            "p t (h c) -> p t h c", c=C),
                        in0=kvg[:, 0:nt, F:KV].rearrange(
                            "p t (h c) -> p t h c", c=C),
                        in1=rhs[:, 0:nt, 0:H].to_broadcast([128, nt, H, C]),
                        op=OP.mult)

                    for i in range(nt):
                        tg = t0 + i
                        f, b, st, sp = tmeta[tg]
                        if st:
                            psum_blk[(f, b)] = pbp.tile(
                                [128, RHSW], F32, name="pblk", tag="pblk")
                        nc.tensor.matmul(
                            psum_blk[(f, b)][:],
                            Sg[:, i * BLK:(i + 1) * BLK],
                            rhs[:, i, :], start=st, stop=sp)
                        if not sp:
                            continue
                        ps = psum_blk.pop((f, b))
                        pa_sl = partA[:, b * RHSW:(b + 1) * RHSW]
                        if f == 0:
                            nc.scalar.copy(pa_sl, ps[:])
                            continue
                        # ---- epilogue for block b ----
                        tot = ep.tile([128, RHSW], F32, tag="tot")
                        nc.vector.tensor_tensor(tot[:], ps[:], pa_sl,
                                                op=OP.add)
                        rec = ep.tile([128, H], F32, tag="rec")
                        nc.vector.scalar_tensor_tensor(
                            out=rec[:], in0=tot[:, 0:H], scalar=float(H),
                            in1=eps2[:], op0=OP.mult, op1=OP.add)
                        nc.vector.reciprocal(rec[:], rec[:])
                        m0 = ep.tile([128, C], F32, tag="m0")
                        nc.vector.scalar_tensor_tensor(
                            out=m0[:], in0=tot[:, H:H + C],
                            scalar=rec[:, 0:1],
                            in1=s_sb[:, b * HID:(b + 1) * HID],
                            op0=OP.mult, op1=OP.add)
                        hp2 = ep.tile([128, HID], F32, tag="hp2")
                        nc.vector.scalar_tensor_tensor(
                            out=hp2[:], in0=tot[:, H + C:H + 2 * C],
                            scalar=rec[:, 1:2], in1=m0[:],
                            op0=OP.mult, op1=OP.add)
                        hblk = ep.tile([128, HID], F32, tag="hblk")
                        nc.scalar.activation(hblk[:], hp2[:], AF.Relu)
                        if layer < 3:
                            pst = epp.tile([HID, 128], F32)
                            nc.tensor.transpose(pst[:], hblk[:], ident[:])
                            nc.scalar.copy(
                                hTown[0:HID, b * 128:(b + 1) * 128], pst[:])
                            cc = col_of_end_blk.get(b)
                            if cc is not None:
                                nc.scalar.dma_start(
                                    coll_in[layer][cc][:],
                                    hTown[:, COFF[cc] * 128:
                                          (COFF[cc] + CW[cc]) * 128])
                        else:
                            nc.sync.dma_start(
                                h_out.ap()[b * 128:(b + 1) * 128, :],
                                hblk[:])

                    for act in sched.pop(jl, []):
                        do_act(act)
                assert not psum_blk
                # boundary: per-column coll then its kv chunks (kv gates the
                # next layer's first gather), qs last (B consumes it early
                # but in block order, so the first tiles suffice)
                for act in boundary:
                    do_act(act)
                if layer < 3:
                    issue_qs(layer + 1)
    nc.compile()
    return nc


# ---------------- public entry ----------------
_CACHE = {}


def _weights_from_inputs(inputs, d):
    """Packed columns: k | v | q' | s'  (WC = 448).

    Folds (exact up to fp rounding; zero-in-degree nodes get the bv shift
    the reference omits — probability ~e^-16 per node, ignored):
      q' = q / sqrt(C)     (alpha = q'.k, no runtime scale)
      bk dropped entirely: q.bk is constant over each softmax group
                           (per dst, head) so it cancels in the softmax
      bv folded into the skip bias:  out += mean_h bv_h
    """
    H, C, F, HID = d["H"], d["C"], d["F"], d["HID"]
    wt = {}
    for L in (1, 2, 3):
        Wk = np.asarray(inputs[f"W{L}k"], np.float32)
        Wv = np.asarray(inputs[f"W{L}v"], np.float32)
        Wq = np.asarray(inputs[f"W{L}q"], np.float32) / np.sqrt(np.float32(C))
        Ws = np.asarray(inputs[f"W{L}s"], np.float32)
        bv = np.asarray(inputs[f"b{L}v"], np.float32)
        bq = np.asarray(inputs[f"b{L}q"], np.float32) / np.sqrt(np.float32(C))
        bs = np.asarray(inputs[f"b{L}s"], np.float32)
        bs2 = bs + bv.reshape(H, C).mean(axis=0)
        W_all = np.concatenate([Wk, Wv, Wq, Ws], axis=1)
        b_all = np.concatenate([np.zeros(2 * F, np.float32), bq, bs2])
        wt[L] = (W_all, b_all)
    return wt


def _install_ntff_shim():
    import types
    if "antenv.axon_hooks" in sys.modules:
        return
    try:
        from trn_agent_boot.trn_boot import _ntff_profile_via_ctypes
        hook = _ntff_profile_via_ctypes("/opt/axon/libaxon_pjrt.so")
    except Exception:
        hook = None
    mod = types.ModuleType("antenv.axon_hooks")
    mod.get_axon_ntff_profile_hook = lambda: hook
    mod.set_axon_ntff_profile_hook = lambda h: None
    sys.modules["antenv.axon_hooks"] = mod
    try:
        import antenv
        antenv.axon_hooks = mod
    except Exception:
        pass


def run(inputs, cfg=SPEC, trace=False):
    d = _derive(cfg)
    wt = _weights_from_inputs(inputs, d)
    in_maps, meta = _prep(inputs["x"], inputs["edge_index"], wt, d)
    key = (tuple(sorted(cfg.items())), meta["TT"],
           tuple(tuple(r) for r in meta["tiles"]))
    if key not in _CACHE:
        _CACHE[key] = build_module(d, meta)
    nc = _CACHE[key]
    if trace:
        _install_ntff_shim()
    res = bass_utils.run_bass_kernel_spmd(
        nc, in_maps, core_ids=list(range(d["M"])), trace=trace)
    node_of_pos = meta["node_of_pos"]
    full = np.zeros((d["N"], d["HID"]), np.float32)
    for m in range(d["M"]):
        out_m = np.asarray(res.results[m]["h_out"], np.float32)
        valid = node_of_pos[m] >= 0
        full[node_of_pos[m][valid]] = out_m[valid]
    return full, res


def kernel(**inputs) -> np.ndarray:
    trace = bool(os.environ.get("KERNEL_TRACE"))
    full, res = run(inputs, SPEC, trace=trace)
    if trace and res.exec_time_ns is not None:
        print(f"HW exec time: {res.exec_time_ns} ns")
    return full


# revision 31
# speedup vs baseline: 1.0258x; 1.0258x over previous
"""Trainium2 Bass kernel for a 3-layer TransformerConv GNN (nn_EncoderTransformerConv).

v2 — flat cross-layer pipeline keyed on keeping the GpSimd gather-descriptor
engine (the serial bottleneck, ~4.8ns per scanned edge slot) busy:

  - Host: nodes are re-assigned to (core, block) bins so per-(core, half,
    block) edge counts are flattened to <=1024 with overflow concentrated in
    the top block index -> tile count drops ~11% (and with it gather-ucode
    time, S/ST bytes, and DVE work).
  - Gathers are issued as SWDGE prepare_only descriptors with a PREP_D-deep
    trigger pipeline: descriptor generation (the Q7 cost) runs ahead while
    triggers carry the kv-table data dependency.
  - Next-layer phase A (kv table recompute) is interleaved into this layer's
    phase B instruction stream: h^T is all-gathered in 7 column chunks as
    dst-block epilogues complete, and kv chunks for the low half are computed
    during the tail of phase B so the next layer's first gather trigger fires
    within ~50us of the last epilogue. kv tables are double-buffered by layer
    parity so writes never WAR against in-flight gathers.
  - Everything else (one-hot fp8 q-broadcast / segmented-reduce matmuls,
    folded biases, bf16 kv rows) follows the v1 design.
"""
import os
import sys
from collections import defaultdict

sys.path.insert(0, "/opt/trn_rl_repo")

import ml_dtypes
import numpy as np

import concourse.bass as bass
import concourse.bacc as bacc
import concourse.mybir as mybir
import concourse.tile as tile
from concourse import bass_utils, library_config
from concourse.masks import make_identity

F32 = mybir.dt.float32
BF16 = mybir.dt.bfloat16
FP8 = mybir.dt.float8e4
I16 = mybir.dt.int16
AF = mybir.ActivationFunctionType
OP = mybir.AluOpType
BNP = ml_dtypes.bfloat16
F8NP = ml_dtypes.float8_e4m3

N_QUEUES = 4          # SWDGE queues; gathers round-robin across them
SPEC = dict(N=50000, E=800000, D_IN=128, HID=64, H=2, M=8)
TILE_E = 128          # edges per tile
CHUNK_T = 20          # tiles per edge-phase chunk
BLK = 128             # dst nodes per block
KVG_BUFS = 4
BAL_LIMIT = 8 * TILE_E   # per-(core,half,block) edge-count target (8 tiles)
COLL_MARGIN = 3       # B-chunks between a block's data chunk and its coll issue
KVHI_START, KVHI_PER = 1, 3  # hi-half kv chunks: 3 per B-chunk from chunk 1


def _derive(cfg):
    d = dict(cfg)
    d["C"] = d["HID"]
    d["F"] = d["H"] * d["C"]            # 128 = q/k/v width
    d["WC"] = 3 * d["F"] + d["HID"]     # 448 packed k|v|q|s
    d["KV"] = 2 * d["F"]                # 256
    d["NPC_REAL"] = d["N"] // d["M"]
    d["NBLK"] = -(-d["NPC_REAL"] // BLK)
    d["NPC"] = d["NBLK"] * BLK
    d["NPAD"] = d["M"] * d["NPC"]
    d["HALF"] = d["NPAD"] // 2
    pa = 1
    for c in range(1, 9):
        if d["NBLK"] % c == 0:
            pa = c
    d["PA_CHUNK"] = pa                  # 7
    d["RANK_CH"] = d["NBLK"] // pa      # 7 phase-A columns per rank
    d["RHSW"] = d["H"] + d["F"]         # 130 = denom cols + exv cols
    return d


def _wrap_idx(a):
    """[M, n] int -> wrapped idx layout [M, 128, n//16] (16-partition wrap,
    replicated to 8 GPSIMD core groups)."""
    Mn, n = a.shape
    w = a.reshape(Mn, n // 16, 16).transpose(0, 2, 1)
    return np.ascontiguousarray(np.tile(w, (1, 8, 1))).astype(np.int16)


def _balance_core(dlo, dhi, nblk, cap, limit):
    """Assign nodes (with per-half in-degrees dlo/dhi) to nblk bins of <=cap
    nodes, flattening both half-sums to <=limit; overflow concentrates in the
    highest bin labels (so tile overflow aligns across cores)."""
    n = len(dlo)
    order = np.argsort(-(dlo + dhi), kind="stable")
    slo = np.zeros(nblk, np.int64)
    shi = np.zeros(nblk, np.int64)
    cnt = np.zeros(nblk, np.int64)
    assign = np.empty(n, np.int64)
    cnt_soft = n / nblk
    for v in order:
        a, b = int(dlo[v]), int(dhi[v])
        ok = (cnt < cap) & (slo + a <= limit) & (shi + b <= limit)
        if ok.any():
            # keep degree AND count pressure flat so the feasible set stays
            # open; the max-term steers away from whichever binds first
            score = np.maximum(
                np.maximum(slo + a, shi + b) / limit,
                (cnt + 1) / cnt_soft)
            score[~ok] = 1e18
            bin_ = int(np.argmin(score))
        else:
            load = (slo + shi).astype(np.float64)
            load[cnt >= cap] = -1e18
            bin_ = int(np.argmax(load))
        assign[v] = bin_
        slo[bin_] += a
        shi[bin_] += b
        cnt[bin_] += 1
    # repair: concentrate residual overshoot into one overflow bin via
    # count-preserving swaps (move a heavy node out of an over-limit bin,
    # a light node in from the overflow bin)
    T = int(np.argmax(slo + shi))
    nodes_by_bin = [list(np.nonzero(assign == b)[0]) for b in range(nblk)]
    for B in range(nblk):
        if B == T:
            continue
        guard = 0
        while (slo[B] > limit or shi[B] > limit) and guard < 200:
            guard += 1
            key = dlo if (slo[B] - limit) >= (shi[B] - limit) else dhi
            vb = max(nodes_by_bin[B], key=lambda v: (key[v], dlo[v] + dhi[v]))
            wt = min(nodes_by_bin[T], key=lambda v: dlo[v] + dhi[v])
            for v, frm, to in ((vb, B, T), (wt, T, B)):
                nodes_by_bin[frm].remove(v)
                nodes_by_bin[to].append(v)
                assign[v] = to
                slo[frm] -= dlo[v]; shi[frm] -= dhi[v]; cnt[frm] -= 1
                slo[to] += dlo[v]; shi[to] += dhi[v]; cnt[to] += 1
    # overflow (most-loaded) bins land at the LOWEST labels so the last
    # blocks processed — which gate the layer-boundary collective — are light
    key = np.maximum(slo, shi)
    relab = np.empty(nblk, np.int64)
    relab[np.argsort(-key, kind="stable")] = np.arange(nblk)
    return relab[assign]


def _prep(x, edge_index, weights, d):
    """Host-side preprocessing -> (in_maps, meta)."""
    M, NPC_REAL, NPC, NPAD, HALF, NBLK = (
        d["M"], d["NPC_REAL"], d["NPC"], d["NPAD"], d["HALF"], d["NBLK"])
    N, D_IN = d["N"], d["D_IN"]

    src = np.asarray(edge_index[0]).astype(np.int64)
    dst = np.asarray(edge_index[1]).astype(np.int64)

    # ---- balanced node -> (core, block) assignment ----
    core_of_node = np.arange(N) // NPC_REAL     # core split kept as-is
    lo_edge = core_of_node[src] < M // 2
    dlo_n = np.bincount(dst[lo_edge], minlength=N)
    dhi_n = np.bincount(dst[~lo_edge], minlength=N)
    pos_of_node = np.empty(N, np.int64)
    node_of_pos = np.full((M, NPC), -1, np.int64)
    for m in range(M):
        ids = np.arange(m * NPC_REAL, (m + 1) * NPC_REAL)
        blk_a = _balance_core(dlo_n[ids], dhi_n[ids], NBLK, BLK, BAL_LIMIT)
        order = np.argsort(blk_a, kind="stable")
        cnts = np.bincount(blk_a, minlength=NBLK)
        offs = np.concatenate([[0], np.cumsum(cnts)])
        ranks = np.arange(NPC_REAL) - offs[blk_a[order]]
        pos_local = blk_a[order] * BLK + ranks
        pos_of_node[ids[order]] = m * NPC + pos_local
        node_of_pos[m, pos_local] = ids[order]

    p_src = pos_of_node[src]
    p_dst = pos_of_node[dst]
    core = p_dst // NPC
    pl = p_dst - core * NPC
    blk = pl // BLK
    dloc_e = pl - blk * BLK
    half = (p_src >= HALF).astype(np.int64)

    counts = np.zeros((M, 2, NBLK), np.int64)
    np.add.at(counts, (core, half, blk), 1)
    tiles = np.maximum(1, -(-counts.max(axis=0) // TILE_E))  # [2, NBLK]
    flat_tiles = tiles.reshape(-1)
    tile_off = np.concatenate([[0], np.cumsum(flat_tiles)])
    TT = int(tile_off[-1])

    # stable-sort edges by (core, half, blk); rank within group
    key = (core * 2 + half) * NBLK + blk
    order = np.argsort(key, kind="stable")
    sk = key[order]
    new_run = np.ones(len(sk), bool)
    new_run[1:] = sk[1:] != sk[:-1]
    run_idx = np.cumsum(new_run) - 1
    starts = np.nonzero(new_run)[0]
    rank = np.arange(len(sk)) - starts[run_idx]
    grp = (half * NBLK + blk)[order]
    pos = tile_off[grp] * TILE_E + rank
    corev = core[order]

    kv_idx = np.zeros((M, TT * TILE_E), np.int64)
    dloc = np.full((M, TT * TILE_E), -1, np.int64)
    kv_idx[corev, pos] = (p_src - half * HALF)[order]
    dloc[corev, pos] = dloc_e[order]
    assert kv_idx.max() < 2 ** 15

    S = np.zeros((M, 128, TT * BLK), F8NP)
    ST = np.zeros((M, 128, TT * BLK), F8NP)
    dd = dloc.reshape(M, TT, TILE_E)
    mm, tt, pp = np.nonzero(dd >= 0)
    dv = dd[mm, tt, pp]
    S[mm, pp, tt * BLK + dv] = 1.0
    ST[mm, dv, tt * BLK + pp] = 1.0

    kv_w = _wrap_idx(kv_idx)

    # node features, transposed + padded (bf16)
    xT_pad = np.zeros((D_IN, NPAD), np.float32)
    xT_pad[:, pos_of_node] = np.asarray(x).T
    xT = xT_pad.astype(BNP)

    wt = {}
    for L in (1, 2, 3):
        W_all, b_all = weights[L]   # packed k|v|q|s by _weights_from_inputs
        if L == 1:
            wt["W1"] = W_all.astype(BNP)
            wt["b1row"] = np.ascontiguousarray(b_all[None, :].astype(BNP))
        else:
            wt[f"W{L}"] = np.concatenate(
                [W_all, b_all[None, :]], 0).astype(BNP)

    in_maps = []
    for m in range(M):
        im = dict(
            xT=np.ascontiguousarray(xT),
            xoT=np.ascontiguousarray(xT[:, m * NPC:(m + 1) * NPC]),
            kvidx=np.ascontiguousarray(kv_w[m]),
            S_in=np.ascontiguousarray(S[m]),
            ST_in=np.ascontiguousarray(ST[m]),
            **wt,
        )
        in_maps.append(im)

    # tile metadata: (half, blk, start, stop)
    meta_t = []
    for f in (0, 1):
        for b in range(NBLK):
            T = int(tiles[f, b])
            for i in range(T):
                meta_t.append((f, b, i == 0, i == T - 1))
    # chunks: runs of <= CHUNK_T tiles within each half; the tail of half 1
    # uses small chunks so the end-of-layer pipeline drains fast
    T0 = int(tiles[0].sum())
    chunks = []
    for f, lo, hi in ((0, 0, T0), (1, T0, TT)):
        t = lo
        while t < hi:
            if f == 1 and hi - t <= 30:
                nt = min(10, hi - t)
            else:
                nt = min(CHUNK_T, hi - t)
            chunks.append((t, nt, f))
            t += nt
    chunk_of_tile = np.empty(TT, np.int64)
    for j, (t0, nt, f) in enumerate(chunks):
        chunk_of_tile[t0:t0 + nt] = j
    # last tile (global) of each half-1 block, for the coll schedule
    blk_end1 = {}
    for b in range(NBLK):
        blk_end1[b] = int(tile_off[NBLK + b] + tiles[1, b] - 1)
    return in_maps, dict(TT=TT, meta=meta_t, chunks=chunks, tiles=tiles,
                         chunk_of_tile=chunk_of_tile, blk_end1=blk_end1,
                         node_of_pos=node_of_pos)


def build_module(d, meta):
    TT, chunks, tmeta = meta["TT"], meta["chunks"], meta["meta"]
    chunk_of_tile, blk_end1 = meta["chunk_of_tile"], meta["blk_end1"]
    M, NPC, NPAD, HALF, NBLK, PA_CHUNK, RANK_CH = (
        d["M"], d["NPC"], d["NPAD"], d["HALF"], d["NBLK"], d["PA_CHUNK"],
        d["RANK_CH"])
    D_IN, F, KV, WC, HID, H, C, RHSW = (
        d["D_IN"], d["F"], d["KV"], d["WC"], d["HID"], d["H"], d["C"],
        d["RHSW"])
    # phase-A columns: 6 of 8 blocks + a final 1-block column, so the
    # layer-boundary collective + kv work (which gate the next layer's first
    # gather) are tiny
    CW = []
    b = NBLK
    while b > 0:
        w = min(8, b) if b != 9 else 8
        CW.append(w)
        b -= w
    COFF = [sum(CW[:i]) for i in range(len(CW))]
    NCOL = len(CW)
    n_ch = len(chunks)

    nc = bacc.Bacc("TRN2", target_bir_lowering=False, debug=False,
                   num_devices=M, num_swdge_queues=N_QUEUES)
    inp = {}
    for name, shape, dt in [
        ("xT", [D_IN, NPAD], BF16), ("xoT", [D_IN, NPC], BF16),
        ("W1", [D_IN, WC], BF16), ("b1row", [1, WC], BF16),
        ("W2", [HID + 1, WC], BF16), ("W3", [HID + 1, WC], BF16),
        ("kvidx", [128, TT * 8], I16),
        ("S_in", [128, TT * BLK], FP8), ("ST_in", [128, TT * BLK], FP8),
    ]:
        inp[name] = nc.dram_tensor(name, shape, dt, kind="ExternalInput")
    h_out = nc.dram_tensor("h_out", [NPC, HID], F32, kind="ExternalOutput")

    with tile.TileContext(nc) as tc:
        with tc.tile_pool(name="dram", bufs=1, space="DRAM") as dram, \
             tc.tile_pool(name="res", bufs=1) as res, \
             tc.tile_pool(name="pa", bufs=4) as pa, \
             tc.tile_pool(name="pap", bufs=2, space="PSUM") as pap, \
             tc.tile_pool(name="pb", bufs=3) as pb, \
             tc.tile_pool(name="pb1", bufs=3) as pb1, \
             tc.tile_pool(name="ep", bufs=2) as ep:

            # kv tables double-buffered by layer parity
            kvt = {par: (dram.tile([HALF, KV], BF16, name=f"kv{par}lo"),
                         dram.tile([NPAD - HALF, KV], BF16, name=f"kv{par}hi"))
                   for par in (0, 1)}
            coll_in = {ly: [dram.tile([HID + 1, CW[c] * 128], BF16,
                                      name=f"ci{ly}_{c}")
                            for c in range(NCOL)] for ly in (1, 2)}
            coll_out = {ly: [dram.tile([M * (HID + 1), CW[c] * 128], BF16,
                                       name=f"co{ly}_{c}")
                             for c in range(NCOL)] for ly in (1, 2)}

            nc.gpsimd.load_library(library_config.mlp)

            # resident SBUF
            W1_sb = res.tile([D_IN, WC], BF16)
            b1row_sb = res.tile([1, WC], BF16)
            ones1_sb = res.tile([1, 128], BF16)
            W2_sb = res.tile([HID + 1, WC], BF16)
            W3_sb = res.tile([HID + 1, WC], BF16)
            kvidx_sb = res.tile([128, TT * 8], I16)
            q_sb = res.tile([128, NBLK * F], BF16)
            s_sb = res.tile([128, NBLK * HID], F32)
            hTown = res.tile([HID + 1, NPC], BF16)
            partA = res.tile([128, NBLK * RHSW], BF16)
            ident = res.tile([128, 128], F32)
            eps2 = res.tile([128, H], F32)

            for sb, t in ((kvidx_sb, "kvidx"), (W1_sb, "W1"),
                          (b1row_sb, "b1row"), (W2_sb, "W2"), (W3_sb, "W3")):
                nc.sync.dma_start(sb[:], inp[t].ap())
            make_identity(nc, ident[:])
            nc.vector.memset(hTown[HID:HID + 1, :], 1.0)
            nc.vector.memset(ones1_sb[:], 1.0)
            nc.vector.memset(eps2[:], H * 1e-16)

            nregs = {}
            for (t0, nt, f) in chunks:
                nn_ = nt * TILE_E
                if nn_ not in nregs:
                    nregs[nn_] = nc.gpsimd.to_reg(nn_)

            # ---------- helpers ----------
            drain_flip = [0]

            def drain(dst_ap, src_ap):
                # 2:1 scalar:vector — DVE is the hotter engine
                if drain_flip[0] % 3 != 2:
                    nc.scalar.copy(dst_ap, src_ap)
                else:
                    nc.vector.tensor_copy(dst_ap, src_ap)
                drain_flip[0] += 1

            pap_cur = [None]  # prologue uses a wider scratch PSUM pool

            def issue_qs_chunk(layer, c):
                W_sb = {2: W2_sb, 3: W3_sb}.get(layer)
                w = CW[c]
                if layer == 1:
                    la = pa.tile([D_IN, w * 128], BF16, tag="la")
                    nc.scalar.dma_start(
                        la[:], inp["xoT"].ap()[
                            :, COFF[c] * 128:(COFF[c] + w) * 128])
                for t in range(w):
                    gt = COFF[c] + t
                    ps = pap_cur[0].tile([128, KV], F32, tag="pakv",
                                         name="paqs")
                    psv = ps[:, 0:WC - KV]
                    if layer == 1:
                        nc.tensor.matmul(psv, la[:, t * 128:(t + 1) * 128],
                                         W1_sb[:, KV:WC],
                                         start=True, stop=False)
                        nc.tensor.matmul(psv, ones1_sb[:],
                                         b1row_sb[:, KV:WC],
                                         start=False, stop=True)
                    else:
                        nc.tensor.matmul(
                            psv, hTown[:, gt * 128:(gt + 1) * 128],
                            W_sb[:, KV:WC], start=True, stop=True)
                    nc.scalar.copy(q_sb[:, gt * F:(gt + 1) * F],
                                   ps[:, 0:F])
                    nc.vector.tensor_copy(
                        s_sb[:, gt * HID:(gt + 1) * HID],
                        ps[:, F:F + HID])

            def issue_qs(layer):
                for c in range(NCOL):
                    issue_qs_chunk(layer, c)

            def issue_kv_chunk(layer, r, c):
                par = layer % 2
                w = CW[c]
                if layer == 1:
                    la = pa.tile([D_IN, w * 128], BF16, tag="la")
                    col0 = r * NPC + COFF[c] * 128
                    nc.scalar.dma_start(
                        la[:], inp["xT"].ap()[:, col0:col0 + w * 128])
                    Wkv = W1_sb
                else:
                    la = pa.tile([HID + 1, w * 128], BF16, tag="la")
                    nc.scalar.dma_start(
                        la[:], coll_out[layer - 1][c][
                            r * (HID + 1):(r + 1) * (HID + 1), :])
                    Wkv = W2_sb if layer == 2 else W3_sb
                kvst = pa.tile([128, w * KV], BF16, tag="kvst")
                for t in range(w):
                    ps = pap_cur[0].tile([128, KV], F32, tag="pakv",
                                         name="pakv")
                    nc.tensor.matmul(ps[:], la[:, t * 128:(t + 1) * 128],
                                     Wkv[:, 0:KV], start=True, stop=True)
                    drain(kvst[:, t * KV:(t + 1) * KV], ps[:])
                row0 = r * NPC + COFF[c] * 128
                kv_lo, kv_hi = kvt[par]
                tgt = (kv_lo[row0:row0 + w * 128, :] if row0 < HALF else
                       kv_hi[row0 - HALF:row0 - HALF + w * 128, :])
                nc.sync.dma_start(
                    tgt.rearrange("(t p) e -> p t e", p=128),
                    kvst[:].rearrange("p (t e) -> p t e", e=KV))

            def issue_coll(layer, c):
                # input dma was issued inline at the source block's epilogue
                nc.gpsimd.collective_compute(
                    "AllGather", OP.bypass,
                    ins=[coll_in[layer][c].opt()],
                    outs=[coll_out[layer][c].opt()],
                    replica_groups=[list(range(M))])

            col_of_end_blk = {COFF[c] + CW[c] - 1: c for c in range(NCOL)}

            def do_act(act):
                if act[0] == "kv":
                    issue_kv_chunk(act[1], act[2], act[3])
                elif act[0] == "coll":
                    issue_coll(act[1], act[2])

            # ---------- layer 1 prologue (qs chunk 0 first; rest follows) ----
            # wide scratch PSUM pool: phase-B PSUM pools are not open yet, so
            # the prologue matmul->drain pipeline gets 6 banks
            with tc.tile_pool(name="pap0", bufs=6, space="PSUM") as pap0:
                pap_cur[0] = pap0
                issue_qs_chunk(1, 0)
                for c in range(NCOL):
                    if c > 0:
                        issue_qs_chunk(1, c)
                    for r in range(M // 2):
                        issue_kv_chunk(1, r, c)
            pap_cur[0] = pap

            # ---------- layer loop: phase B with interleaved next phase A ----
            with tc.tile_pool(name="pbp", bufs=2, space="PSUM") as pbp, \
                 tc.tile_pool(name="qep", bufs=2, space="PSUM") as qep, \
                 tc.tile_pool(name="epp", bufs=2, space="PSUM") as epp:
              for layer in (1, 2, 3):
                par = layer % 2
                kv_lo, kv_hi = kvt[par]

                sched = defaultdict(list)
                # hi-half kv chunks of THIS layer into early B chunks
                hi_jobs = [("kv", layer, r, c)
                           for c in range(NCOL)
                           for r in range(M // 2, M)]
                jj = KVHI_START
                while hi_jobs:
                    for _ in range(KVHI_PER):
                        if hi_jobs:
                            sched[jj].append(hi_jobs.pop(0))
                    jj += 1
                boundary = []
                if layer < 3:
                    for c in range(NCOL):
                        jc = int(chunk_of_tile[
                            blk_end1[COFF[c] + CW[c] - 1]]) + COLL_MARGIN
                        acts = [("coll", layer, c)] + [
                            ("kv", layer + 1, r, c) for r in range(M // 2)]
                        if jc < n_ch:
                            sched[jc] += acts
                        else:
                            boundary += acts

                psum_blk = {}
                for jl, (t0, nt, fhalf) in enumerate(chunks):
                    n = nt * TILE_E
                    kvg = pb.tile([128, CHUNK_T, KV], BF16, tag="kvg",
                                  bufs=5)
                    qg = pb.tile([128, CHUNK_T, F], BF16, tag="qg")
                    Sg = pb1.tile([128, CHUNK_T * BLK], FP8, tag="Sg")
                    STg = pb1.tile([128, CHUNK_T * BLK], FP8, tag="STg")
                    prod = pb1.tile([128, CHUNK_T * F], BF16, tag="prod",
                                    bufs=2)
                    alph = pb1.tile([128, CHUNK_T * H], F32, tag="alph")
                    rhs = pb.tile([128, CHUNK_T, RHSW], BF16, tag="rhs",
                                  bufs=2)

                    in_ap = kv_lo[:] if fhalf == 0 else kv_hi[:]
                    nc.gpsimd.dma_gather(
                        out_ap=kvg[:, 0:nt, :], in_ap=in_ap,
                        idxs_ap=kvidx_sb[:, t0 * 8:t0 * 8 + nt * 8],
                        num_idxs=n, num_idxs_reg=nregs[n], elem_size=KV,
                        single_packet=False, queue_num=jl % N_QUEUES)
                    nc.sync.dma_start(
                        STg[:, 0:n],
                        inp["ST_in"].ap()[:, t0 * BLK:t0 * BLK + n])
                    nc.sync.dma_start(
                        Sg[:, 0:n],
                        inp["S_in"].ap()[:, t0 * BLK:t0 * BLK + n])
                    # q at edges via one-hot matmul; PSUM drained to bf16
                    for i in range(nt):
                        b = tmeta[t0 + i][1]
                        qe = qep.tile([128, F], F32, name="qe", tag="qe")
                        nc.tensor.matmul(
                            qe[:], STg[:, i * BLK:(i + 1) * BLK],
                            q_sb[:, b * F:(b + 1) * F],
                            start=True, stop=True)
                        if i % 3 != 2:
                            nc.scalar.copy(qg[:, i, :], qe[:])
                        else:
                            nc.vector.tensor_copy(qg[:, i, :], qe[:])

                    # chunk-wide: prod = q'_e * k_e (bf16, 2x DVE mode)
                    nc.vector.tensor_tensor(
                        out=prod[:, 0:nt * F].rearrange(
                            "p (t f) -> p t f", f=F),
                        in0=qg[:, 0:nt, :], in1=kvg[:, 0:nt, 0:F], op=OP.mult)
                    nc.vector.reduce_sum(
                        alph[:, 0:nt * H],
                        prod[:, 0:nt * F].rearrange(
                            "p (th c) -> p th c", c=C),
                        axis=mybir.AxisListType.X)
                    nc.scalar.activation(
                        rhs[:, 0:nt, 0:H],
                        alph[:, 0:nt * H].rearrange("p (t h) -> p t h", h=H),
                        AF.Exp)
                    nc.vector.tensor_tensor(
                        out=rhs[:, 0:nt, H:RHSW].rearrange(
                            "p t (h c) -> p t h c", c=C),
                        in0=kvg[:, 0:nt, F:KV].rearrange(
                            "p t (h c) -> p t h c", c=C),
                        in1=rhs[:, 0:nt, 0:H].to_broadcast([128, nt, H, C]),
                        op=OP.mult)

                    for i in range(nt):
                        tg = t0 + i
                        f, b, st, sp = tmeta[tg]
                        if st:
                            psum_blk[(f, b)] = pbp.tile(
                                [128, RHSW], F32, name="pblk", tag="pblk")
                        nc.tensor.matmul(
                            psum_blk[(f, b)][:],
                            Sg[:, i * BLK:(i + 1) * BLK],
                            rhs[:, i, :], start=st, stop=sp)
                        if not sp:
                            continue
                        ps = psum_blk.pop((f, b))
                        pa_sl = partA[:, b * RHSW:(b + 1) * RHSW]
                        if f == 0:
                            nc.scalar.copy(pa_sl, ps[:])
                            continue
                        # ---- epilogue for block b ----
                        tot = ep.tile([128, RHSW], F32, tag="tot")
                        nc.vector.tensor_tensor(tot[:], ps[:], pa_sl,
                                                op=OP.add)
                        rec = ep.tile([128, H], F32, tag="rec")
                        nc.vector.scalar_tensor_tensor(
                            out=rec[:], in0=tot[:, 0:H], scalar=float(H),
                            in1=eps2[:], op0=OP.mult, op1=OP.add)
                        nc.vector.reciprocal(rec[:], rec[:])
                        m0 = ep.tile([128, C], F32, tag="m0")
                        nc.vector.scalar_tensor_tensor(
                            out=m0[:], in0=tot[:, H:H + C],
                            scalar=rec[:, 0:1],
                            in1=s_sb[:, b * HID:(b + 1) * HID],
                            op0=OP.mult, op1=OP.add)
                        hp2 = ep.tile([128, HID], F32, tag="hp2")
                        nc.vector.scalar_tensor_tensor(
                            out=hp2[:], in0=tot[:, H + C:H + 2 * C],
                            scalar=rec[:, 1:2], in1=m0[:],
                            op0=OP.mult, op1=OP.add)
                        hblk = ep.tile([128, HID], F32, tag="hblk")
                        nc.scalar.activation(hblk[:], hp2[:], AF.Relu)
                        if layer < 3:
                            pst = epp.tile([HID, 128], F32)
                            nc.tensor.transpose(pst[:], hblk[:], ident[:])
                            nc.scalar.copy(
                                hTown[0:HID, b * 128:(b + 1) * 128], pst[:])
                            cc = col_of_end_blk.get(b)
                            if cc is not None:
                                nc.scalar.dma_start(
                                    coll_in[layer][cc][:],
                                    hTown[:, COFF[cc] * 128:
                                          (COFF[cc] + CW[cc]) * 128])
                        else:
                            nc.sync.dma_start(
                                h_out.ap()[b * 128:(b + 1) * 128, :],
                                hblk[:])

                    for act in sched.pop(jl, []):
                        do_act(act)
                assert not psum_blk
                # boundary: per-column coll then its kv chunks (kv gates the
                # next layer's first gather), qs last (B consumes it early
                # but in block order, so the first tiles suffice)
                for act in boundary:
                    do_act(act)
                if layer < 3:
                    issue_qs(layer + 1)
    nc.compile()
    return nc


# ---------------- public entry ----------------
_CACHE = {}


def _weights_from_inputs(inputs, d):
    """Packed columns: k | v | q' | s'  (WC = 448).

    Folds (exact up to fp rounding; zero-in-degree nodes get the bv shift
    the reference omits — probability ~e^-16 per node, ignored):
      q' = q / sqrt(C)     (alpha = q'.k, no runtime scale)
      bk dropped entirely: q.bk is constant over each softmax group
                           (per dst, head) so it cancels in the softmax
      bv folded into the skip bias:  out += mean_h bv_h
    """
    H, C, F, HID = d["H"], d["C"], d["F"], d["HID"]
    wt = {}
    for L in (1, 2, 3):
        Wk = np.asarray(inputs[f"W{L}k"], np.float32)
        Wv = np.asarray(inputs[f"W{L}v"], np.float32)
        Wq = np.asarray(inputs[f"W{L}q"], np.float32) / np.sqrt(np.float32(C))
        Ws = np.asarray(inputs[f"W{L}s"], np.float32)
        bv = np.asarray(inputs[f"b{L}v"], np.float32)
        bq = np.asarray(inputs[f"b{L}q"], np.float32) / np.sqrt(np.float32(C))
        bs = np.asarray(inputs[f"b{L}s"], np.float32)
        bs2 = bs + bv.reshape(H, C).mean(axis=0)
        W_all = np.concatenate([Wk, Wv, Wq, Ws], axis=1)
        b_all = np.concatenate([np.zeros(2 * F, np.float32), bq, bs2])
        wt[L] = (W_all, b_all)
    return wt


def _install_ntff_shim():
    import types
    if "antenv.axon_hooks" in sys.modules:
        return
    try:
        from trn_agent_boot.trn_boot import _ntff_profile_via_ctypes
        hook = _ntff_profile_via_ctypes("/opt/axon/libaxon_pjrt.so")
    except Exception:
        hook = None
    mod = types.ModuleType("antenv.axon_hooks")
    mod.get_axon_ntff_profile_hook = lambda: hook
    mod.set_axon_ntff_profile_hook = lambda h: None
    sys.modules["antenv.axon_hooks"] = mod
    try:
        import antenv
        antenv.axon_hooks = mod
    except Exception:
        pass


def run(inputs, cfg=SPEC, trace=False):
    d = _derive(cfg)
    wt = _weights_from_inputs(inputs, d)
    in_maps, meta = _prep(inputs["x"], inputs["edge_index"], wt, d)
    key = (tuple(sorted(cfg.items())), meta["TT"],
           tuple(tuple(r) for r in meta["tiles"]))
    if key not in _CACHE:
        _CACHE[key] = build_module(d, meta)
    nc = _CACHE[key]
    if trace:
        _install_ntff_shim()
    res = bass_utils.run_bass_kernel_spmd(
        nc, in_maps, core_ids=list(range(d["M"])), trace=trace)
    node_of_pos = meta["node_of_pos"]
    full = np.zeros((d["N"], d["HID"]), np.float32)
    for m in range(d["M"]):
        out_m = np.asarray(res.results[m]["h_out"], np.float32)
        valid = node_of_pos[m] >= 0
        full[node_of_pos[m][valid]] = out_m[valid]
    return full, res


def kernel(**inputs) -> np.ndarray:
    trace = bool(os.environ.get("KERNEL_TRACE"))
    full, res = run(inputs, SPEC, trace=trace)
    if trace and res.exec_time_ns is not None:
        print(f"HW exec time: {res.exec_time_ns} ns")
    return full
